# revision 15
# baseline (speedup 1.0000x reference)
"""Trainium2 Bass kernel for two-stream cross-attention (v7).

Reference computation (per batch b):
    qkv_s = x_s @ W_qkv_s ; split into q_s, k_s, v_s (16 heads x 64)
    dir1: out1 = softmax(q2 k1^T * scale) v1, merged @ W_out1 + b_out1
    dir2: out2 = softmax(q1 k2^T * scale) v2, merged @ W_out2 + b_out2

Sharding: 8 cores = 2 batches x 4 head-groups (4 heads each). Each core
computes q/k/v for its 4 heads (both streams), both attention directions,
and a partial output projection (row-block of W_out). Host transposes x
(so the device only does linear DMA) and sums the 4 f16 partials per
batch, adding the bias.

Kernel structure (all matmuls bf16, fp32 PSUM accumulation):
  - Heads processed in row-tiled PAIRS: head 2*cb on partitions 0-63,
    head 2*cb+1 on 64-127. The pair's two S^T matmuls (K=64) carry
    tile_position (0,0)/(64,0), run CONCURRENTLY in the PE array into
    different PSUM banks -> 2x S throughput.
  - Flash-style inner loop per key block: S-pair -> one exp[128,1024]
    covering both heads -> 8 AV matmuls accumulating into per-head
    oav[128,4,72] PSUM banks (appended ones-column = softmax rowsum).
  - ScalarE (exp) is the critical engine (~285us busy). The PE's
    attention work (small AV matmuls, transposes, outproj) doubles as
    HAM activity keeping the PE clock at 2.4 GHz; a leaner AV structure
    (v6) measured WORSE because the PE throttled to 1.2 GHz and became
    the bottleneck.
  - v7 head: only dir0-critical DMA early (x1+x2, w1 k/v cols, w2 q
    cols; dir1 w cols and wo deferred), and the FIRST attention unit is
    striped into the kT0/v0 fill chunks so exp starts as soon as the
    x DMA lands instead of after all fills.
  - PSUM pools: st 4 banks, oav 2 (attention-only), ptr/outproj+head
    fills 1, pmm fills 1.
"""

import os

import numpy as np
import ml_dtypes

import concourse.bass as bass
import concourse.mybir as mybir
import concourse.tile as tile
from concourse import bacc
from concourse.bass_utils import run_bass_kernel_spmd
from concourse.masks import make_identity

BF16 = mybir.dt.bfloat16
F16 = mybir.dt.float16
F32 = mybir.dt.float32


B, N, DIM = 2, 2048, 1024
HEADS, DH = 16, 64
HPC = 4                      # heads per core
HC = HPC * DH                # 256 inner columns per core
SCALE = DH ** -0.5
P = 128
FB = DIM // P                # 8 feature blocks
KB = N // P                  # 16 key blocks
QT = 512                     # q-tile
NQT = N // QT                # 4 q-tiles
NM = QT // P                 # 4 m-blocks per q-tile

NCORES = 8
LOWPRI = 10_000_000          # negative high_priority offset for fill work

_NC = None
LAST_RESULTS = None


def _build():
    nc = bacc.Bacc(None, target_bir_lowering=False, debug=False, num_devices=NCORES)

    # x is pre-transposed on the host: x^T [DIM, N] -> linear DMA loads
    xs = [nc.dram_tensor(f"x{s + 1}", [DIM, N], BF16, kind="ExternalInput")
          for s in range(2)]
    ws = [nc.dram_tensor(f"w{s + 1}", [DIM, 3 * HC], BF16, kind="ExternalInput")
          for s in range(2)]
    wos = [nc.dram_tensor(f"wo{s + 1}", [HC, DIM], BF16, kind="ExternalInput")
           for s in range(2)]
    os_ = [nc.dram_tensor(f"o{d + 1}", [N, DIM], F16, kind="ExternalOutput")
           for d in range(2)]

    with tile.TileContext(nc) as tc:
        with (
            tc.tile_pool(name="const", bufs=1) as const_pool,
            tc.tile_pool(name="qkv", bufs=1) as qkv_pool,
        ):
            identity = const_pool.tile([P, P], BF16)
            make_identity(nc, identity[:])
            wo_sb = [const_pool.tile([P, 2, DIM], BF16, name=f"wo{d}")
                     for d in range(2)]

            # persistent per-stream q/k/v (bf16) and per-dir O^T
            qT = [qkv_pool.tile([P, 2, N], BF16, name=f"qT{s}") for s in range(2)]
            kT = [qkv_pool.tile([P, 2, N], BF16, name=f"kT{s}") for s in range(2)]
            vx = [qkv_pool.tile([P, KB, HPC, DH + 1], BF16, name=f"vx{s}")
                  for s in range(2)]
            ot = [qkv_pool.tile([P, 2, N], BF16, name=f"ot{d}") for d in range(2)]
            for s in range(2):
                nc.vector.memset(vx[s][:, :, :, DH], 1.0)

            with (
                tc.tile_pool(name="xT", bufs=1) as xt_pool,
                tc.tile_pool(name="wsb", bufs=1) as w_pool,
                tc.tile_pool(name="pmm", bufs=1, space="PSUM") as pmm_pool,
                tc.tile_pool(name="st", bufs=2, space="PSUM") as st_pool,
                tc.tile_pool(name="oav", bufs=2, space="PSUM") as oav_pool,
                tc.tile_pool(name="ptrpop", bufs=1, space="PSUM") as ptr_pool,
                tc.tile_pool(name="pt", bufs=6) as pt_pool,
                tc.tile_pool(name="osb", bufs=4) as osb_pool,
                tc.tile_pool(name="rec", bufs=4) as rec_pool,
                tc.tile_pool(name="ost", bufs=3) as ost_pool,
            ):
                # HAM warmup: no-dep dummy matmuls, first in the PE queue.
                # The critical x DMA takes ~30us; the first fill group
                # completes only after its full-DIM contraction, so the
                # PE needs dummy coverage through the whole DMA window or
                # the fills run at the cold 1.2 GHz clock.
                warm = pmm_pool.tile([P, P], F32, name="warm", tag="pmm")
                for _ in range(480):
                    nc.tensor.matmul(warm[:], identity[:], identity[:],
                                     start=True, stop=True)

                xT = [xt_pool.tile([P, FB, N], BF16, name=f"xT{s}")
                      for s in range(2)]
                w_sb = [w_pool.tile([P, FB, 3 * HC], BF16, name=f"w{s}")
                        for s in range(2)]
                # Critical DMA first: x of both streams + only the w
                # columns dir0 needs (stream1 k/v, stream2 q). dir1's w
                # columns and wo queue up behind them.
                W_EARLY = [(HC, 3 * HC), (0, HC)]
                W_LATE = [(0, HC), (HC, 3 * HC)]
                for fb in range(FB):
                    for s in (1, 0):
                        lo, hi = W_EARLY[s]
                        nc.sync.dma_start(
                            w_sb[s][:, fb, lo:hi],
                            ws[s][fb * P:(fb + 1) * P, lo:hi])
                        nc.sync.dma_start(
                            xT[s][:, fb, :], xs[s][fb * P:(fb + 1) * P, :])
                for fb in range(FB):
                    for s in (1, 0):
                        lo, hi = W_LATE[s]
                        nc.sync.dma_start(
                            w_sb[s][:, fb, lo:hi],
                            ws[s][fb * P:(fb + 1) * P, lo:hi])
                for d in range(2):
                    for cb in range(2):
                        nc.sync.dma_start(
                            wo_sb[d][:, cb, :], wos[d][cb * P:(cb + 1) * P, :])

                def qk_group(s, off, cb, nt, dest, pool, tag):
                    ps = pool.tile([P, 512], F32, name="pqk", tag=tag)
                    for fb in range(FB):
                        nc.tensor.matmul(
                            ps[:],
                            w_sb[s][:, fb, off + cb * P:off + (cb + 1) * P],
                            xT[s][:, fb, nt * 512:(nt + 1) * 512],
                            start=(fb == 0), stop=(fb == FB - 1))
                    nc.vector.tensor_copy(dest[:, cb, nt * 512:(nt + 1) * 512],
                                          ps[:])

                def v_group(s, kb, pool, tag):
                    ps = pool.tile([P, HC], F32, name="pv", tag=tag)
                    for fb in range(FB):
                        nc.tensor.matmul(
                            ps[:],
                            xT[s][:, fb, kb * P:(kb + 1) * P],
                            w_sb[s][:, fb, 2 * HC:3 * HC],
                            start=(fb == 0), stop=(fb == FB - 1))
                    nc.vector.tensor_copy(
                        vx[s][:, kb, :, 0:DH],
                        ps[:].rearrange("p (h d) -> p h d", h=HPC))

                def attn_begin():
                    return [oav_pool.tile([P, NM, 72], F32, name="oav",
                                          tag="oav")
                            for _ in range(2)]

                def attn_chunk(oav, d, qs, ks, qt, cb, kb_lo, kb_hi,
                               spice=None):
                    q_t, k_t, v_t = qT[qs], kT[ks], vx[ks]
                    q0 = qt * QT
                    for kb in range(kb_lo, kb_hi):
                        st = st_pool.tile([P, 2, QT], F32, name="st", tag="st")
                        for hh in range(2):
                            po = hh * DH
                            nc.tensor.matmul(
                                st[:, hh, :],
                                k_t[po:po + DH, cb, kb * P:(kb + 1) * P],
                                q_t[po:po + DH, cb, q0:q0 + QT],
                                start=True, stop=True)
                        pt = pt_pool.tile([P, 2, QT], BF16, name="pt")
                        nc.scalar.activation(
                            pt[:], st[:],
                            mybir.ActivationFunctionType.Exp, scale=SCALE)
                        for hh in range(2):
                            head = 2 * cb + hh
                            for m in range(NM):
                                nc.tensor.matmul(
                                    oav[hh][:, m, 0:DH + 1],
                                    pt[:, hh, m * P:(m + 1) * P],
                                    v_t[:, kb, head, :],
                                    start=(kb == 0 and m == 0),
                                    stop=(kb == KB - 1 and m == NM - 1),
                                    skip_group_check=True)
                        if spice is not None:
                            # HAM feeder: keep the PE clock warm through
                            # the exp-paced end phase (fills exhausted)
                            for _ in range(4):
                                nc.tensor.matmul(spice[:], identity[:],
                                                 identity[:],
                                                 start=True, stop=True)

                def attn_end(oav, d, qt, cb):
                    # normalize, transpose O -> O^T, write into ot.
                    # Mildly deprioritized (far above the fill stream)
                    # so the scheduler starts the NEXT unit's S/exp
                    # before these transposes instead of blocking the
                    # scalar pipeline at every unit boundary.
                    q0 = qt * QT
                    with tc.high_priority(offset=-1000):
                        ptr = ptr_pool.tile([DH, 2 * NM, P], BF16, name="ptr",
                                            tag="ptrpop")
                        for hh in range(2):
                            for m in range(NM):
                                rec = rec_pool.tile([P, 1], F32, name="rec")
                                nc.vector.reciprocal(rec[:],
                                                     oav[hh][:, m, DH:DH + 1])
                                osb = osb_pool.tile([P, DH], BF16, name="osb")
                                nc.vector.tensor_scalar_mul(
                                    osb[:], oav[hh][:, m, 0:DH], rec[:])
                                nc.tensor.transpose(
                                    ptr[:, hh * NM + m, :], osb[:], identity[:])
                        for hh in range(2):
                            po = hh * DH
                            nc.vector.tensor_copy(
                                ot[d][po:po + DH, cb, q0:q0 + QT],
                                ptr[:, hh * NM:(hh + 1) * NM, :])

                def attn_unit(d, qs, ks, qt, cb):
                    oav = attn_begin()
                    attn_chunk(oav, d, qs, ks, qt, cb, 0, KB)
                    attn_end(oav, d, qt, cb)

                def outproj(d, qt, twobank=False):
                    for mb in range(NM):
                        row = qt * QT + mb * P
                        ost = ost_pool.tile([P, DIM], F16, name="ost")
                        for nb in range(2):
                            # late outprojs: the fill bank is free, use it
                            # to pipeline the pops 2-deep
                            if twobank and (mb * 2 + nb) % 2 == 1:
                                pop = pmm_pool.tile([P, 512], F32, name="pop",
                                                    tag="pmm")
                            else:
                                pop = ptr_pool.tile([P, 512], F32, name="pop",
                                                    tag="ptrpop")
                            for cb2 in range(2):
                                nc.tensor.matmul(
                                    pop[:],
                                    ot[d][:, cb2, row:row + P],
                                    wo_sb[d][:, cb2, nb * 512:(nb + 1) * 512],
                                    start=(cb2 == 0), stop=(cb2 == 1))
                            nc.vector.tensor_copy(ost[:, nb * 512:(nb + 1) * 512],
                                                  pop[:])
                        nc.sync.dma_start(os_[d][row:row + P, :], ost[:])

                # ---- head: dir0's first unit striped into its fills ----
                # Fill groups alternate the pmm/ptrpop banks (2-deep
                # pipeline); st/oav stay attention-only. After each
                # kT0-nt + v0 chunk, the matching kb chunk of unit
                # (dir0, qt0, cb0) is emitted, so exp starts right after
                # the x DMA + ~4 fill groups instead of after all 26.
                hseq = [(pmm_pool, "pmm"), (ptr_pool, "ptrpop")]
                hidx = [0]

                def nxt():
                    pl = hseq[hidx[0] % 2]
                    hidx[0] += 1
                    return pl

                # The 6 PSUM bank-slots that attention will use later
                # (st x2, oav x2) are idle during the DMA window; route
                # the first 6 fill groups through them so all 6
                # pre-accumulate per-fb as x lands, instead of
                # serializing through a 2-bank pipeline afterwards.
                qk_group(1, 0, 0, 0, qT[1], pmm_pool, "pmm")
                qk_group(1, 0, 1, 0, qT[1], ptr_pool, "ptrpop")
                qk_group(0, HC, 0, 0, kT[0], st_pool, "st")
                qk_group(0, HC, 1, 0, kT[0], st_pool, "st")
                v_group(0, 0, oav_pool, "oav")
                v_group(0, 1, oav_pool, "oav")
                oav0 = attn_begin()
                for nt in range(4):
                    if nt > 0:
                        for cb in range(2):
                            pl = nxt()
                            qk_group(0, HC, cb, nt, kT[0], pl[0], pl[1])
                    for kb in range(4 * nt + (2 if nt == 0 else 0),
                                    4 * nt + 4):
                        pl = nxt()
                        v_group(0, kb, pl[0], pl[1])
                    attn_chunk(oav0, 0, 1, 0, 0, 0, 4 * nt, 4 * nt + 4)
                attn_end(oav0, 0, 0, 0)

                # ---- remaining qkv at very low scheduler preference:
                # executes only in PE gaps of the Scalar-bound attention
                # stream. Ordered by first use: dir0 qt1's queries, then
                # dir1 nt0 k/q, dir0 qt2/3 queries, dir1 v + rest.
                with tc.high_priority(offset=-LOWPRI):
                    for cb in range(2):
                        qk_group(1, 0, cb, 1, qT[1], pmm_pool, "pmm")
                    for cb in range(2):
                        qk_group(0, 0, cb, 0, qT[0], pmm_pool, "pmm")
                        qk_group(1, HC, cb, 0, kT[1], pmm_pool, "pmm")
                    for nt in range(2, 4):
                        for cb in range(2):
                            qk_group(1, 0, cb, nt, qT[1], pmm_pool, "pmm")
                    for nt in range(4):
                        for kb in range(4 * nt, 4 * nt + 4):
                            v_group(1, kb, pmm_pool, "pmm")
                        if nt > 0:
                            for cb in range(2):
                                qk_group(1, HC, cb, nt, kT[1], pmm_pool, "pmm")
                    for nt in range(1, 4):
                        for cb in range(2):
                            qk_group(0, 0, cb, nt, qT[0], pmm_pool, "pmm")

                # ---- attention + inline output projection ----
                attn_unit(0, 1, 0, 0, 1)
                outproj(0, 0)
                for qt in range(1, NQT):
                    for cb in range(2):
                        attn_unit(0, 1, 0, qt, cb)
                    outproj(0, qt)
                for qt in range(NQT):
                    for cb in range(2):
                        if qt >= 2:
                            warm2 = pmm_pool.tile([P, P], F32, name="warm2",
                                                  tag="pmm")
                            oavL = attn_begin()
                            attn_chunk(oavL, 1, 0, 1, qt, cb, 0, KB,
                                       spice=warm2)
                            attn_end(oavL, 1, qt, cb)
                        else:
                            attn_unit(1, 0, 1, qt, cb)
                    outproj(1, qt, twobank=(qt >= 2))

    nc.compile()
    return nc


def _shard_inputs(x1, x2, W_qkv1, W_qkv2, W_out1, W_out2):
    bf = ml_dtypes.bfloat16
    in_maps = []
    xs = [np.ascontiguousarray(x1).astype(bf), np.ascontiguousarray(x2).astype(bf)]
    w_full = [np.asarray(W_qkv1), np.asarray(W_qkv2)]
    wo_full = [np.asarray(W_out1), np.asarray(W_out2)]
    for cid in range(NCORES):
        b, g = divmod(cid, 4)
        cs = slice(g * HC, (g + 1) * HC)
        m = {}
        for s in range(2):
            m[f"x{s + 1}"] = np.ascontiguousarray(xs[s][b].T)
            w = w_full[s]
            m[f"w{s + 1}"] = np.ascontiguousarray(np.concatenate(
                [w[:, 0:DIM][:, cs], w[:, DIM:2 * DIM][:, cs],
                 w[:, 2 * DIM:3 * DIM][:, cs]], axis=1)).astype(bf)
            m[f"wo{s + 1}"] = np.ascontiguousarray(wo_full[s][cs, :]).astype(bf)
        in_maps.append(m)
    return in_maps


def kernel(x1, x2, W_qkv1, W_qkv2, W_out1, b_out1, W_out2, b_out2):
    global _NC, LAST_RESULTS
    if _NC is None:
        _NC = _build()

    in_maps = _shard_inputs(x1, x2, W_qkv1, W_qkv2, W_out1, W_out2)
    trace = bool(os.environ.get("BASS_KERNEL_TRACE"))
    res = run_bass_kernel_spmd(_NC, in_maps, list(range(NCORES)), trace=trace)
    LAST_RESULTS = res

    out1 = np.zeros((B, N, DIM), np.float32)
    out2 = np.zeros((B, N, DIM), np.float32)
    for cid in range(NCORES):
        b = cid // 4
        out1[b] += res.results[cid]["o1"].astype(np.float32)
        out2[b] += res.results[cid]["o2"].astype(np.float32)
    out1 += np.asarray(b_out1, np.float32)
    out2 += np.asarray(b_out2, np.float32)
    return out1, out2


# revision 16
# speedup vs baseline: 1.0003x; 1.0003x over previous
"""Trainium2 Bass kernel for two-stream cross-attention (v7).

Reference computation (per batch b):
    qkv_s = x_s @ W_qkv_s ; split into q_s, k_s, v_s (16 heads x 64)
    dir1: out1 = softmax(q2 k1^T * scale) v1, merged @ W_out1 + b_out1
    dir2: out2 = softmax(q1 k2^T * scale) v2, merged @ W_out2 + b_out2

Sharding: 8 cores = 2 batches x 4 head-groups (4 heads each). Each core
computes q/k/v for its 4 heads (both streams), both attention directions,
and a partial output projection (row-block of W_out). Host transposes x
(so the device only does linear DMA) and sums the 4 f16 partials per
batch, adding the bias.

Kernel structure (all matmuls bf16, fp32 PSUM accumulation):
  - Heads processed in row-tiled PAIRS: head 2*cb on partitions 0-63,
    head 2*cb+1 on 64-127. The pair's two S^T matmuls (K=64) carry
    tile_position (0,0)/(64,0), run CONCURRENTLY in the PE array into
    different PSUM banks -> 2x S throughput.
  - Flash-style inner loop per key block: S-pair -> one exp[128,1024]
    covering both heads -> 8 AV matmuls accumulating into per-head
    oav[128,4,72] PSUM banks (appended ones-column = softmax rowsum).
  - ScalarE (exp) is the critical engine (~285us busy). The PE's
    attention work (small AV matmuls, transposes, outproj) doubles as
    HAM activity keeping the PE clock at 2.4 GHz; a leaner AV structure
    (v6) measured WORSE because the PE throttled to 1.2 GHz and became
    the bottleneck.
  - v7 head: only dir0-critical DMA early (x1+x2, w1 k/v cols, w2 q
    cols; dir1 w cols and wo deferred), and the FIRST attention unit is
    striped into the kT0/v0 fill chunks so exp starts as soon as the
    x DMA lands instead of after all fills.
  - PSUM pools: st 4 banks, oav 2 (attention-only), ptr/outproj+head
    fills 1, pmm fills 1.
"""

import os

import numpy as np
import ml_dtypes

import concourse.bass as bass
import concourse.mybir as mybir
import concourse.tile as tile
from concourse import bacc
from concourse.bass_utils import run_bass_kernel_spmd
from concourse.masks import make_identity

BF16 = mybir.dt.bfloat16
F16 = mybir.dt.float16
F32 = mybir.dt.float32


B, N, DIM = 2, 2048, 1024
HEADS, DH = 16, 64
HPC = 4                      # heads per core
HC = HPC * DH                # 256 inner columns per core
SCALE = DH ** -0.5
P = 128
FB = DIM // P                # 8 feature blocks
KB = N // P                  # 16 key blocks
QT = 512                     # q-tile
NQT = N // QT                # 4 q-tiles
NM = QT // P                 # 4 m-blocks per q-tile

NCORES = 8
LOWPRI = 10_000_000          # negative high_priority offset for fill work

_NC = None
LAST_RESULTS = None


def _build():
    nc = bacc.Bacc(None, target_bir_lowering=False, debug=False, num_devices=NCORES)

    # x is pre-transposed on the host: x^T [DIM, N] -> linear DMA loads
    xs = [nc.dram_tensor(f"x{s + 1}", [DIM, N], BF16, kind="ExternalInput")
          for s in range(2)]
    ws = [nc.dram_tensor(f"w{s + 1}", [DIM, 3 * HC], BF16, kind="ExternalInput")
          for s in range(2)]
    wos = [nc.dram_tensor(f"wo{s + 1}", [HC, DIM], BF16, kind="ExternalInput")
           for s in range(2)]
    os_ = [nc.dram_tensor(f"o{d + 1}", [N, DIM], F16, kind="ExternalOutput")
           for d in range(2)]

    with tile.TileContext(nc) as tc:
        with (
            tc.tile_pool(name="const", bufs=1) as const_pool,
            tc.tile_pool(name="qkv", bufs=1) as qkv_pool,
        ):
            identity = const_pool.tile([P, P], BF16)
            make_identity(nc, identity[:])
            wo_sb = [const_pool.tile([P, 2, DIM], BF16, name=f"wo{d}")
                     for d in range(2)]

            # persistent per-stream q/k/v (bf16) and per-dir O^T
            qT = [qkv_pool.tile([P, 2, N], BF16, name=f"qT{s}") for s in range(2)]
            kT = [qkv_pool.tile([P, 2, N], BF16, name=f"kT{s}") for s in range(2)]
            vx = [qkv_pool.tile([P, KB, HPC, DH + 1], BF16, name=f"vx{s}")
                  for s in range(2)]
            ot = [qkv_pool.tile([P, 2, N], BF16, name=f"ot{d}") for d in range(2)]
            for s in range(2):
                nc.vector.memset(vx[s][:, :, :, DH], 1.0)

            with (
                tc.tile_pool(name="xT", bufs=1) as xt_pool,
                tc.tile_pool(name="wsb", bufs=1) as w_pool,
                tc.tile_pool(name="pmm", bufs=1, space="PSUM") as pmm_pool,
                tc.tile_pool(name="st", bufs=2, space="PSUM") as st_pool,
                tc.tile_pool(name="oav", bufs=2, space="PSUM") as oav_pool,
                tc.tile_pool(name="ptrpop", bufs=1, space="PSUM") as ptr_pool,
                tc.tile_pool(name="pt", bufs=6) as pt_pool,
                tc.tile_pool(name="osb", bufs=4) as osb_pool,
                tc.tile_pool(name="rec", bufs=4) as rec_pool,
                tc.tile_pool(name="ost", bufs=3) as ost_pool,
            ):
                # HAM warmup: no-dep dummy matmuls, first in the PE queue.
                # The critical x DMA takes ~30us; the first fill group
                # completes only after its full-DIM contraction, so the
                # PE needs dummy coverage through the whole DMA window or
                # the fills run at the cold 1.2 GHz clock.
                warm = pmm_pool.tile([P, P], F32, name="warm", tag="pmm")
                for _ in range(480):
                    nc.tensor.matmul(warm[:], identity[:], identity[:],
                                     start=True, stop=True)

                xT = [xt_pool.tile([P, FB, N], BF16, name=f"xT{s}")
                      for s in range(2)]
                w_sb = [w_pool.tile([P, FB, 3 * HC], BF16, name=f"w{s}")
                        for s in range(2)]
                # Critical DMA first: x of both streams + only the w
                # columns dir0 needs (stream1 k/v, stream2 q). dir1's w
                # columns and wo queue up behind them.
                W_EARLY = [(HC, 3 * HC), (0, HC)]
                W_LATE = [(0, HC), (HC, 3 * HC)]
                for fb in range(FB):
                    for s in (1, 0):
                        lo, hi = W_EARLY[s]
                        nc.sync.dma_start(
                            w_sb[s][:, fb, lo:hi],
                            ws[s][fb * P:(fb + 1) * P, lo:hi])
                        nc.sync.dma_start(
                            xT[s][:, fb, :], xs[s][fb * P:(fb + 1) * P, :])
                for fb in range(FB):
                    for s in (1, 0):
                        lo, hi = W_LATE[s]
                        nc.sync.dma_start(
                            w_sb[s][:, fb, lo:hi],
                            ws[s][fb * P:(fb + 1) * P, lo:hi])
                for d in range(2):
                    for cb in range(2):
                        nc.sync.dma_start(
                            wo_sb[d][:, cb, :], wos[d][cb * P:(cb + 1) * P, :])

                def qk_group(s, off, cb, nt, dest, pool, tag):
                    ps = pool.tile([P, 512], F32, name="pqk", tag=tag)
                    for fb in range(FB):
                        nc.tensor.matmul(
                            ps[:],
                            w_sb[s][:, fb, off + cb * P:off + (cb + 1) * P],
                            xT[s][:, fb, nt * 512:(nt + 1) * 512],
                            start=(fb == 0), stop=(fb == FB - 1))
                    nc.vector.tensor_copy(dest[:, cb, nt * 512:(nt + 1) * 512],
                                          ps[:])

                def v_group(s, kb, pool, tag):
                    ps = pool.tile([P, HC], F32, name="pv", tag=tag)
                    for fb in range(FB):
                        nc.tensor.matmul(
                            ps[:],
                            xT[s][:, fb, kb * P:(kb + 1) * P],
                            w_sb[s][:, fb, 2 * HC:3 * HC],
                            start=(fb == 0), stop=(fb == FB - 1))
                    nc.vector.tensor_copy(
                        vx[s][:, kb, :, 0:DH],
                        ps[:].rearrange("p (h d) -> p h d", h=HPC))

                def attn_begin():
                    return [oav_pool.tile([P, NM, 72], F32, name="oav",
                                          tag="oav")
                            for _ in range(2)]

                def attn_chunk(oav, d, qs, ks, qt, cb, kb_lo, kb_hi,
                               spice=None):
                    q_t, k_t, v_t = qT[qs], kT[ks], vx[ks]
                    q0 = qt * QT
                    for kb in range(kb_lo, kb_hi):
                        st = st_pool.tile([P, 2, QT], F32, name="st", tag="st")
                        for hh in range(2):
                            po = hh * DH
                            nc.tensor.matmul(
                                st[:, hh, :],
                                k_t[po:po + DH, cb, kb * P:(kb + 1) * P],
                                q_t[po:po + DH, cb, q0:q0 + QT],
                                start=True, stop=True)
                        pt = pt_pool.tile([P, 2, QT], BF16, name="pt")
                        nc.scalar.activation(
                            pt[:], st[:],
                            mybir.ActivationFunctionType.Exp, scale=SCALE)
                        for hh in range(2):
                            head = 2 * cb + hh
                            for m in range(NM):
                                nc.tensor.matmul(
                                    oav[hh][:, m, 0:DH + 1],
                                    pt[:, hh, m * P:(m + 1) * P],
                                    v_t[:, kb, head, :],
                                    start=(kb == 0 and m == 0),
                                    stop=(kb == KB - 1 and m == NM - 1),
                                    skip_group_check=True)
                        if spice is not None:
                            # HAM feeder: keep the PE clock warm through
                            # the exp-paced end phase (fills exhausted)
                            for _ in range(4):
                                nc.tensor.matmul(spice[:], identity[:],
                                                 identity[:],
                                                 start=True, stop=True)

                def attn_end(oav, d, qt, cb):
                    # normalize, transpose O -> O^T, write into ot
                    q0 = qt * QT
                    ptr = ptr_pool.tile([DH, 2 * NM, P], BF16, name="ptr",
                                        tag="ptrpop")
                    for hh in range(2):
                        for m in range(NM):
                            rec = rec_pool.tile([P, 1], F32, name="rec")
                            nc.vector.reciprocal(rec[:], oav[hh][:, m, DH:DH + 1])
                            osb = osb_pool.tile([P, DH], BF16, name="osb")
                            nc.vector.tensor_scalar_mul(
                                osb[:], oav[hh][:, m, 0:DH], rec[:])
                            nc.tensor.transpose(
                                ptr[:, hh * NM + m, :], osb[:], identity[:])
                    for hh in range(2):
                        po = hh * DH
                        nc.vector.tensor_copy(
                            ot[d][po:po + DH, cb, q0:q0 + QT],
                            ptr[:, hh * NM:(hh + 1) * NM, :])

                def attn_unit(d, qs, ks, qt, cb):
                    oav = attn_begin()
                    attn_chunk(oav, d, qs, ks, qt, cb, 0, KB)
                    attn_end(oav, d, qt, cb)

                def outproj(d, qt, twobank=False):
                    for mb in range(NM):
                        row = qt * QT + mb * P
                        ost = ost_pool.tile([P, DIM], F16, name="ost")
                        for nb in range(2):
                            # late outprojs: the fill bank is free, use it
                            # to pipeline the pops 2-deep
                            if twobank and (mb * 2 + nb) % 2 == 1:
                                pop = pmm_pool.tile([P, 512], F32, name="pop",
                                                    tag="pmm")
                            else:
                                pop = ptr_pool.tile([P, 512], F32, name="pop",
                                                    tag="ptrpop")
                            for cb2 in range(2):
                                nc.tensor.matmul(
                                    pop[:],
                                    ot[d][:, cb2, row:row + P],
                                    wo_sb[d][:, cb2, nb * 512:(nb + 1) * 512],
                                    start=(cb2 == 0), stop=(cb2 == 1))
                            nc.vector.tensor_copy(ost[:, nb * 512:(nb + 1) * 512],
                                                  pop[:])
                        nc.sync.dma_start(os_[d][row:row + P, :], ost[:])

                # ---- head: dir0's first unit striped into its fills ----
                # Fill groups alternate the pmm/ptrpop banks (2-deep
                # pipeline); st/oav stay attention-only. After each
                # kT0-nt + v0 chunk, the matching kb chunk of unit
                # (dir0, qt0, cb0) is emitted, so exp starts right after
                # the x DMA + ~4 fill groups instead of after all 26.
                hseq = [(pmm_pool, "pmm"), (ptr_pool, "ptrpop")]
                hidx = [0]

                def nxt():
                    pl = hseq[hidx[0] % 2]
                    hidx[0] += 1
                    return pl

                # The 6 PSUM bank-slots that attention will use later
                # (st x2, oav x2) are idle during the DMA window; route
                # the first 6 fill groups through them so all 6
                # pre-accumulate per-fb as x lands, instead of
                # serializing through a 2-bank pipeline afterwards.
                qk_group(1, 0, 0, 0, qT[1], pmm_pool, "pmm")
                qk_group(1, 0, 1, 0, qT[1], ptr_pool, "ptrpop")
                qk_group(0, HC, 0, 0, kT[0], st_pool, "st")
                qk_group(0, HC, 1, 0, kT[0], st_pool, "st")
                v_group(0, 0, oav_pool, "oav")
                v_group(0, 1, oav_pool, "oav")
                oav0 = attn_begin()
                for nt in range(4):
                    if nt > 0:
                        for cb in range(2):
                            pl = nxt()
                            qk_group(0, HC, cb, nt, kT[0], pl[0], pl[1])
                    for kb in range(4 * nt + (2 if nt == 0 else 0),
                                    4 * nt + 4):
                        pl = nxt()
                        v_group(0, kb, pl[0], pl[1])
                    attn_chunk(oav0, 0, 1, 0, 0, 0, 4 * nt, 4 * nt + 4)
                attn_end(oav0, 0, 0, 0)

                # ---- remaining qkv at very low scheduler preference:
                # executes only in PE gaps of the Scalar-bound attention
                # stream. Ordered by first use: dir0 qt1's queries, then
                # dir1 nt0 k/q, dir0 qt2/3 queries, dir1 v + rest.
                with tc.high_priority(offset=-LOWPRI):
                    for cb in range(2):
                        qk_group(1, 0, cb, 1, qT[1], pmm_pool, "pmm")
                    for cb in range(2):
                        qk_group(0, 0, cb, 0, qT[0], pmm_pool, "pmm")
                        qk_group(1, HC, cb, 0, kT[1], pmm_pool, "pmm")
                    for nt in range(2, 4):
                        for cb in range(2):
                            qk_group(1, 0, cb, nt, qT[1], pmm_pool, "pmm")
                    for nt in range(4):
                        for kb in range(4 * nt, 4 * nt + 4):
                            v_group(1, kb, pmm_pool, "pmm")
                        if nt > 0:
                            for cb in range(2):
                                qk_group(1, HC, cb, nt, kT[1], pmm_pool, "pmm")
                    for nt in range(1, 4):
                        for cb in range(2):
                            qk_group(0, 0, cb, nt, qT[0], pmm_pool, "pmm")

                # ---- attention + inline output projection ----
                attn_unit(0, 1, 0, 0, 1)
                outproj(0, 0)
                for qt in range(1, NQT):
                    for cb in range(2):
                        attn_unit(0, 1, 0, qt, cb)
                    outproj(0, qt)
                for qt in range(NQT):
                    for cb in range(2):
                        if qt >= 2:
                            warm2 = pmm_pool.tile([P, P], F32, name="warm2",
                                                  tag="pmm")
                            oavL = attn_begin()
                            attn_chunk(oavL, 1, 0, 1, qt, cb, 0, KB,
                                       spice=warm2)
                            attn_end(oavL, 1, qt, cb)
                        else:
                            attn_unit(1, 0, 1, qt, cb)
                    outproj(1, qt, twobank=(qt >= 2))

    nc.compile()
    return nc


def _shard_inputs(x1, x2, W_qkv1, W_qkv2, W_out1, W_out2):
    bf = ml_dtypes.bfloat16
    in_maps = []
    xs = [np.ascontiguousarray(x1).astype(bf), np.ascontiguousarray(x2).astype(bf)]
    w_full = [np.asarray(W_qkv1), np.asarray(W_qkv2)]
    wo_full = [np.asarray(W_out1), np.asarray(W_out2)]
    for cid in range(NCORES):
        b, g = divmod(cid, 4)
        cs = slice(g * HC, (g + 1) * HC)
        m = {}
        for s in range(2):
            m[f"x{s + 1}"] = np.ascontiguousarray(xs[s][b].T)
            w = w_full[s]
            m[f"w{s + 1}"] = np.ascontiguousarray(np.concatenate(
                [w[:, 0:DIM][:, cs], w[:, DIM:2 * DIM][:, cs],
                 w[:, 2 * DIM:3 * DIM][:, cs]], axis=1)).astype(bf)
            m[f"wo{s + 1}"] = np.ascontiguousarray(wo_full[s][cs, :]).astype(bf)
        in_maps.append(m)
    return in_maps


def kernel(x1, x2, W_qkv1, W_qkv2, W_out1, b_out1, W_out2, b_out2):
    global _NC, LAST_RESULTS
    if _NC is None:
        _NC = _build()

    in_maps = _shard_inputs(x1, x2, W_qkv1, W_qkv2, W_out1, W_out2)
    trace = bool(os.environ.get("BASS_KERNEL_TRACE"))
    res = run_bass_kernel_spmd(_NC, in_maps, list(range(NCORES)), trace=trace)
    LAST_RESULTS = res

    out1 = np.zeros((B, N, DIM), np.float32)
    out2 = np.zeros((B, N, DIM), np.float32)
    for cid in range(NCORES):
        b = cid // 4
        out1[b] += res.results[cid]["o1"].astype(np.float32)
        out2[b] += res.results[cid]["o2"].astype(np.float32)
    out1 += np.asarray(b_out1, np.float32)
    out2 += np.asarray(b_out2, np.float32)
    return out1, out2


# revision 18
# speedup vs baseline: 1.0153x; 1.0150x over previous
"""Trainium2 Bass kernel for two-stream cross-attention (v12).

Reference computation (per batch b):
    qkv_s = x_s @ W_qkv_s ; split into q_s, k_s, v_s (16 heads x 64)
    dir1: out1 = softmax(q2 k1^T * scale) v1, merged @ W_out1 + b_out1
    dir2: out2 = softmax(q1 k2^T * scale) v2, merged @ W_out2 + b_out2

Sharding: 8 cores = 2 batches x 4 head-groups (4 heads each). Each core
computes q/k/v for its 4 heads (both streams), both attention directions,
and a partial output projection (row-block of W_out). Host transposes x
(so the device only does linear DMA) and sums the 4 f16 partials per
batch, adding the bias.

Kernel structure (all matmuls bf16, fp32 PSUM accumulation):
  - Heads processed in row-tiled PAIRS: head 2*cb on partitions 0-63,
    head 2*cb+1 on 64-127. The pair's two S^T matmuls (K=64) carry
    tile_position (0,0)/(64,0), run CONCURRENTLY in the PE array into
    different PSUM banks -> 2x S throughput.
  - Flash-style inner loop per key block: S-pair -> one exp[128,1024]
    covering both heads -> 8 AV matmuls accumulating into per-head
    oav[128,4,72] PSUM banks (appended ones-column = softmax rowsum).
  - ScalarE (exp) is the critical engine: 256 ACTIVATEs x 1113ns =
    285us busy; measured wall ~402us = 46us head + ~337us exp window
    (62us gaps) + 25us tail.
  - HAM WARNING: the PE's "wasteful" small AV matmuls and transposes
    keep the PE clock at 2.4 GHz. A leaner v-stationary AV variant
    (half the PE work) measured 580us, not faster: the PE throttled to
    1.2 GHz (HAM) and became the bottleneck. Keep PE duty high.
  - Head: 480 warmup matmuls cover the ~30us DMA window (else fills
    run at the cold clock); only dir0-critical DMA goes first (x1+x2,
    w1 k/v cols, w2 q cols; dir1 w cols + wo deferred); the first 6
    fill groups pre-accumulate in the 6 idle attention PSUM banks; the
    first attention unit is striped into the kT0/v0 fill chunks.
  - Tail: dir1 qt>=2 units carry 4 dummy matmuls per key block (HAM
    feeder) and their outproj pipelines pops through 2 PSUM banks.
  - fp8 does NOT work here: quantizing W_qkv (x32 prescale) gives
    relmax ~0.077, and even fp8 x with on-device bf16 upconvert gives
    ~0.055 (vs 0.02 gate) - the per-channel/per-token quantization
    error is systematic and does not diffuse over the softmax.
  - PSUM pools: st 4 banks, oav 2 (attention-only), ptr/outproj+head
    fills 1, pmm fills 1.
"""

import os

import numpy as np
import ml_dtypes

import concourse.bass as bass
import concourse.mybir as mybir
import concourse.tile as tile
from concourse import bacc
from concourse.bass_utils import run_bass_kernel_spmd
from concourse.masks import make_identity

BF16 = mybir.dt.bfloat16
F16 = mybir.dt.float16
F32 = mybir.dt.float32


B, N, DIM = 2, 2048, 1024
HEADS, DH = 16, 64
HPC = 4                      # heads per core
HC = HPC * DH                # 256 inner columns per core
SCALE = DH ** -0.5
P = 128
FB = DIM // P                # 8 feature blocks
KB = N // P                  # 16 key blocks
QT = 512                     # q-tile
NQT = N // QT                # 4 q-tiles
NM = QT // P                 # 4 m-blocks per q-tile

NCORES = 8
LOWPRI = 10_000_000          # negative high_priority offset for fill work

_NC = None
LAST_RESULTS = None


def _build():
    nc = bacc.Bacc(None, target_bir_lowering=False, debug=False, num_devices=NCORES)

    # x is pre-transposed on the host: x^T [DIM, N] -> linear DMA loads
    xs = [nc.dram_tensor(f"x{s + 1}", [DIM, N], BF16, kind="ExternalInput")
          for s in range(2)]
    ws = [nc.dram_tensor(f"w{s + 1}", [DIM, 3 * HC], BF16, kind="ExternalInput")
          for s in range(2)]
    wos = [nc.dram_tensor(f"wo{s + 1}", [HC, DIM], BF16, kind="ExternalInput")
           for s in range(2)]
    os_ = [nc.dram_tensor(f"o{d + 1}", [N, DIM], F16, kind="ExternalOutput")
           for d in range(2)]

    with tile.TileContext(nc) as tc:
        with (
            tc.tile_pool(name="const", bufs=1) as const_pool,
            tc.tile_pool(name="qkv", bufs=1) as qkv_pool,
        ):
            identity = const_pool.tile([P, P], BF16)
            make_identity(nc, identity[:])
            wo_sb = [const_pool.tile([P, 2, DIM], BF16, name=f"wo{d}")
                     for d in range(2)]

            # persistent per-stream q/k/v (bf16) and per-dir O^T
            qT = [qkv_pool.tile([P, 2, N], BF16, name=f"qT{s}") for s in range(2)]
            kT = [qkv_pool.tile([P, 2, N], BF16, name=f"kT{s}") for s in range(2)]
            vx = [qkv_pool.tile([P, KB, HPC, DH + 1], BF16, name=f"vx{s}")
                  for s in range(2)]
            ot = [qkv_pool.tile([P, 2, N], BF16, name=f"ot{d}") for d in range(2)]
            for s in range(2):
                nc.vector.memset(vx[s][:, :, :, DH], 1.0)

            with (
                tc.tile_pool(name="xT", bufs=1) as xt_pool,
                tc.tile_pool(name="wsb", bufs=1) as w_pool,
                tc.tile_pool(name="pmm", bufs=1, space="PSUM") as pmm_pool,
                tc.tile_pool(name="st", bufs=2, space="PSUM") as st_pool,
                tc.tile_pool(name="oav", bufs=2, space="PSUM") as oav_pool,
                tc.tile_pool(name="ptrpop", bufs=1, space="PSUM") as ptr_pool,
                tc.tile_pool(name="pt", bufs=6) as pt_pool,
                tc.tile_pool(name="osb", bufs=4) as osb_pool,
                tc.tile_pool(name="rec", bufs=4) as rec_pool,
                tc.tile_pool(name="ost", bufs=3) as ost_pool,
            ):
                # HAM warmup: no-dep dummy matmuls, first in the PE queue.
                # The critical x DMA takes ~30us; the first fill group
                # completes only after its full-DIM contraction, so the
                # PE needs dummy coverage through the whole DMA window or
                # the fills run at the cold 1.2 GHz clock.
                warm = pmm_pool.tile([P, P], F32, name="warm", tag="pmm")
                for _ in range(480):
                    nc.tensor.matmul(warm[:], identity[:], identity[:],
                                     start=True, stop=True)

                xT = [xt_pool.tile([P, FB, N], BF16, name=f"xT{s}")
                      for s in range(2)]
                w_sb = [w_pool.tile([P, FB, 3 * HC], BF16, name=f"w{s}")
                        for s in range(2)]
                # Critical DMA first: x of both streams + only the w
                # columns dir0 needs (stream1 k/v, stream2 q). dir1's w
                # columns and wo queue up behind them.
                W_EARLY = [(HC, 3 * HC), (0, HC)]
                W_LATE = [(0, HC), (HC, 3 * HC)]
                for fb in range(FB):
                    for s in (1, 0):
                        lo, hi = W_EARLY[s]
                        nc.sync.dma_start(
                            w_sb[s][:, fb, lo:hi],
                            ws[s][fb * P:(fb + 1) * P, lo:hi])
                        nc.sync.dma_start(
                            xT[s][:, fb, :], xs[s][fb * P:(fb + 1) * P, :])
                for fb in range(FB):
                    for s in (1, 0):
                        lo, hi = W_LATE[s]
                        nc.sync.dma_start(
                            w_sb[s][:, fb, lo:hi],
                            ws[s][fb * P:(fb + 1) * P, lo:hi])
                for d in range(2):
                    for cb in range(2):
                        nc.sync.dma_start(
                            wo_sb[d][:, cb, :], wos[d][cb * P:(cb + 1) * P, :])

                def qk_group(s, off, cb, nt, dest, pool, tag):
                    ps = pool.tile([P, 512], F32, name="pqk", tag=tag)
                    for fb in range(FB):
                        nc.tensor.matmul(
                            ps[:],
                            w_sb[s][:, fb, off + cb * P:off + (cb + 1) * P],
                            xT[s][:, fb, nt * 512:(nt + 1) * 512],
                            start=(fb == 0), stop=(fb == FB - 1))
                    nc.vector.tensor_copy(dest[:, cb, nt * 512:(nt + 1) * 512],
                                          ps[:])

                def v_group(s, kb, pool, tag):
                    ps = pool.tile([P, HC], F32, name="pv", tag=tag)
                    for fb in range(FB):
                        nc.tensor.matmul(
                            ps[:],
                            xT[s][:, fb, kb * P:(kb + 1) * P],
                            w_sb[s][:, fb, 2 * HC:3 * HC],
                            start=(fb == 0), stop=(fb == FB - 1))
                    nc.vector.tensor_copy(
                        vx[s][:, kb, :, 0:DH],
                        ps[:].rearrange("p (h d) -> p h d", h=HPC))

                def attn_begin():
                    return [oav_pool.tile([P, NM, 72], F32, name="oav",
                                          tag="oav")
                            for _ in range(2)]

                def attn_chunk(oav, d, qs, ks, qt, cb, kb_lo, kb_hi,
                               spice=None):
                    q_t, k_t, v_t = qT[qs], kT[ks], vx[ks]
                    q0 = qt * QT
                    for kb in range(kb_lo, kb_hi):
                        st = st_pool.tile([P, 2, QT], F32, name="st", tag="st")
                        for hh in range(2):
                            po = hh * DH
                            nc.tensor.matmul(
                                st[:, hh, :],
                                k_t[po:po + DH, cb, kb * P:(kb + 1) * P],
                                q_t[po:po + DH, cb, q0:q0 + QT],
                                start=True, stop=True)
                        pt = pt_pool.tile([P, 2, QT], BF16, name="pt")
                        nc.scalar.activation(
                            pt[:], st[:],
                            mybir.ActivationFunctionType.Exp, scale=SCALE)
                        for hh in range(2):
                            head = 2 * cb + hh
                            for m in range(NM):
                                nc.tensor.matmul(
                                    oav[hh][:, m, 0:DH + 1],
                                    pt[:, hh, m * P:(m + 1) * P],
                                    v_t[:, kb, head, :],
                                    start=(kb == 0 and m == 0),
                                    stop=(kb == KB - 1 and m == NM - 1),
                                    skip_group_check=True)
                        if spice is not None:
                            # HAM feeder: keep the PE clock warm through
                            # the exp-paced end phase (fills exhausted)
                            for _ in range(4):
                                nc.tensor.matmul(spice[:], identity[:],
                                                 identity[:],
                                                 start=True, stop=True)

                def attn_end(oav, d, qt, cb):
                    # normalize, transpose O -> O^T, write into ot
                    q0 = qt * QT
                    ptr = ptr_pool.tile([DH, 2 * NM, P], BF16, name="ptr",
                                        tag="ptrpop")
                    for hh in range(2):
                        for m in range(NM):
                            rec = rec_pool.tile([P, 1], F32, name="rec")
                            nc.vector.reciprocal(rec[:], oav[hh][:, m, DH:DH + 1])
                            osb = osb_pool.tile([P, DH], BF16, name="osb")
                            nc.vector.tensor_scalar_mul(
                                osb[:], oav[hh][:, m, 0:DH], rec[:])
                            nc.tensor.transpose(
                                ptr[:, hh * NM + m, :], osb[:], identity[:])
                    for hh in range(2):
                        po = hh * DH
                        nc.vector.tensor_copy(
                            ot[d][po:po + DH, cb, q0:q0 + QT],
                            ptr[:, hh * NM:(hh + 1) * NM, :])

                def attn_unit(d, qs, ks, qt, cb):
                    oav = attn_begin()
                    attn_chunk(oav, d, qs, ks, qt, cb, 0, KB)
                    attn_end(oav, d, qt, cb)

                def outproj(d, qt, twobank=False):
                    for mb in range(NM):
                        row = qt * QT + mb * P
                        ost = ost_pool.tile([P, DIM], F16, name="ost")
                        for nb in range(2):
                            # late outprojs: the fill bank is free, use it
                            # to pipeline the pops 2-deep
                            if twobank and (mb * 2 + nb) % 2 == 1:
                                pop = pmm_pool.tile([P, 512], F32, name="pop",
                                                    tag="pmm")
                            else:
                                pop = ptr_pool.tile([P, 512], F32, name="pop",
                                                    tag="ptrpop")
                            for cb2 in range(2):
                                nc.tensor.matmul(
                                    pop[:],
                                    ot[d][:, cb2, row:row + P],
                                    wo_sb[d][:, cb2, nb * 512:(nb + 1) * 512],
                                    start=(cb2 == 0), stop=(cb2 == 1))
                            nc.vector.tensor_copy(ost[:, nb * 512:(nb + 1) * 512],
                                                  pop[:])
                        nc.sync.dma_start(os_[d][row:row + P, :], ost[:])

                # ---- head: dir0's first unit striped into its fills ----
                # Fill groups alternate the pmm/ptrpop banks (2-deep
                # pipeline); st/oav stay attention-only. After each
                # kT0-nt + v0 chunk, the matching kb chunk of unit
                # (dir0, qt0, cb0) is emitted, so exp starts right after
                # the x DMA + ~4 fill groups instead of after all 26.
                hseq = [(pmm_pool, "pmm"), (ptr_pool, "ptrpop")]
                hidx = [0]

                def nxt():
                    pl = hseq[hidx[0] % 2]
                    hidx[0] += 1
                    return pl

                # The 6 PSUM bank-slots that attention will use later
                # (st x2, oav x2) are idle during the DMA window; route
                # the first 6 fill groups through them so all 6
                # pre-accumulate per-fb as x lands, instead of
                # serializing through a 2-bank pipeline afterwards.
                qk_group(1, 0, 0, 0, qT[1], pmm_pool, "pmm")
                qk_group(1, 0, 1, 0, qT[1], ptr_pool, "ptrpop")
                qk_group(0, HC, 0, 0, kT[0], st_pool, "st")
                qk_group(0, HC, 1, 0, kT[0], st_pool, "st")
                v_group(0, 0, oav_pool, "oav")
                v_group(0, 1, oav_pool, "oav")
                oav0 = attn_begin()
                for nt in range(4):
                    if nt > 0:
                        for cb in range(2):
                            pl = nxt()
                            qk_group(0, HC, cb, nt, kT[0], pl[0], pl[1])
                    for kb in range(4 * nt + (2 if nt == 0 else 0),
                                    4 * nt + 4):
                        pl = nxt()
                        v_group(0, kb, pl[0], pl[1])
                    attn_chunk(oav0, 0, 1, 0, 0, 0, 4 * nt, 4 * nt + 4)
                attn_end(oav0, 0, 0, 0)

                # ---- remaining qkv at very low scheduler preference:
                # executes only in PE gaps of the Scalar-bound attention
                # stream. Ordered by first use: dir0 qt1's queries, then
                # dir1 nt0 k/q, dir0 qt2/3 queries, dir1 v + rest.
                with tc.high_priority(offset=-LOWPRI):
                    for cb in range(2):
                        qk_group(1, 0, cb, 1, qT[1], pmm_pool, "pmm")
                    for cb in range(2):
                        qk_group(0, 0, cb, 0, qT[0], pmm_pool, "pmm")
                        qk_group(1, HC, cb, 0, kT[1], pmm_pool, "pmm")
                    for nt in range(2, 4):
                        for cb in range(2):
                            qk_group(1, 0, cb, nt, qT[1], pmm_pool, "pmm")
                    for nt in range(4):
                        for kb in range(4 * nt, 4 * nt + 4):
                            v_group(1, kb, pmm_pool, "pmm")
                        if nt > 0:
                            for cb in range(2):
                                qk_group(1, HC, cb, nt, kT[1], pmm_pool, "pmm")
                    for nt in range(1, 4):
                        for cb in range(2):
                            qk_group(0, 0, cb, nt, qT[0], pmm_pool, "pmm")

                # ---- attention + inline output projection ----
                def spiced(qt, cb):
                    warm2 = pmm_pool.tile([P, P], F32, name="warm2",
                                          tag="pmm")
                    oavL = attn_begin()
                    attn_chunk(oavL, 1, 0, 1, qt, cb, 0, KB, spice=warm2)
                    attn_end(oavL, 1, qt, cb)

                # Each qt's outproj is deferred until after the NEXT
                # qt's first unit: its 8 pop matmuls then execute in
                # that unit's exp-paced PE slack instead of jamming the
                # scalar pipeline at the qt boundary.
                attn_unit(0, 1, 0, 0, 1)
                attn_unit(0, 1, 0, 1, 0)
                outproj(0, 0)
                attn_unit(0, 1, 0, 1, 1)
                attn_unit(0, 1, 0, 2, 0)
                outproj(0, 1)
                attn_unit(0, 1, 0, 2, 1)
                attn_unit(0, 1, 0, 3, 0)
                outproj(0, 2)
                attn_unit(0, 1, 0, 3, 1)
                outproj(0, 3)
                attn_unit(1, 0, 1, 0, 0)
                attn_unit(1, 0, 1, 0, 1)
                attn_unit(1, 0, 1, 1, 0)
                outproj(1, 0)
                attn_unit(1, 0, 1, 1, 1)
                spiced(2, 0)
                outproj(1, 1)
                spiced(2, 1)
                spiced(3, 0)
                outproj(1, 2)
                spiced(3, 1)
                outproj(1, 3, twobank=True)

    nc.compile()
    return nc


def _shard_inputs(x1, x2, W_qkv1, W_qkv2, W_out1, W_out2):
    bf = ml_dtypes.bfloat16
    in_maps = []
    xs = [np.ascontiguousarray(x1).astype(bf), np.ascontiguousarray(x2).astype(bf)]
    w_full = [np.asarray(W_qkv1), np.asarray(W_qkv2)]
    wo_full = [np.asarray(W_out1), np.asarray(W_out2)]
    for cid in range(NCORES):
        b, g = divmod(cid, 4)
        cs = slice(g * HC, (g + 1) * HC)
        m = {}
        for s in range(2):
            m[f"x{s + 1}"] = np.ascontiguousarray(xs[s][b].T)
            w = w_full[s]
            m[f"w{s + 1}"] = np.ascontiguousarray(np.concatenate(
                [w[:, 0:DIM][:, cs], w[:, DIM:2 * DIM][:, cs],
                 w[:, 2 * DIM:3 * DIM][:, cs]], axis=1)).astype(bf)
            m[f"wo{s + 1}"] = np.ascontiguousarray(wo_full[s][cs, :]).astype(bf)
        in_maps.append(m)
    return in_maps


def kernel(x1, x2, W_qkv1, W_qkv2, W_out1, b_out1, W_out2, b_out2):
    global _NC, LAST_RESULTS
    if _NC is None:
        _NC = _build()

    in_maps = _shard_inputs(x1, x2, W_qkv1, W_qkv2, W_out1, W_out2)
    trace = bool(os.environ.get("BASS_KERNEL_TRACE"))
    res = run_bass_kernel_spmd(_NC, in_maps, list(range(NCORES)), trace=trace)
    LAST_RESULTS = res

    out1 = np.zeros((B, N, DIM), np.float32)
    out2 = np.zeros((B, N, DIM), np.float32)
    for cid in range(NCORES):
        b = cid // 4
        out1[b] += res.results[cid]["o1"].astype(np.float32)
        out2[b] += res.results[cid]["o2"].astype(np.float32)
    out1 += np.asarray(b_out1, np.float32)
    out2 += np.asarray(b_out2, np.float32)
    return out1, out2


# revision 19
# speedup vs baseline: 1.0176x; 1.0023x over previous
"""Trainium2 Bass kernel for two-stream cross-attention (v14).

Reference computation (per batch b):
    qkv_s = x_s @ W_qkv_s ; split into q_s, k_s, v_s (16 heads x 64)
    dir1: out1 = softmax(q2 k1^T * scale) v1, merged @ W_out1 + b_out1
    dir2: out2 = softmax(q1 k2^T * scale) v2, merged @ W_out2 + b_out2

Sharding: 8 cores = 2 batches x 4 head-groups (4 heads each). Each core
computes q/k/v for its 4 heads (both streams), both attention directions,
and a partial output projection (row-block of W_out). Host transposes x
(so the device only does linear DMA) and sums the 4 f16 partials per
batch, adding the bias.

Kernel structure (all matmuls bf16, fp32 PSUM accumulation):
  - Heads processed in row-tiled PAIRS: head 2*cb on partitions 0-63,
    head 2*cb+1 on 64-127. The pair's two S^T matmuls (K=64) carry
    tile_position (0,0)/(64,0), run CONCURRENTLY in the PE array into
    different PSUM banks -> 2x S throughput.
  - Flash-style inner loop per key block: S-pair -> one exp[128,1024]
    covering both heads -> 8 AV matmuls accumulating into per-head
    oav[128,4,72] PSUM banks (appended ones-column = softmax rowsum).
  - ScalarE (exp) is the critical engine: 256 ACTIVATEs x 1113ns =
    285us busy; measured wall ~402us = 46us head + ~337us exp window
    (62us gaps) + 25us tail.
  - HAM WARNING: the PE's "wasteful" small AV matmuls and transposes
    keep the PE clock at 2.4 GHz. A leaner v-stationary AV variant
    (half the PE work) measured 580us, not faster: the PE throttled to
    1.2 GHz (HAM) and became the bottleneck. Keep PE duty high.
  - Head: 480 warmup matmuls cover the ~30us DMA window (else fills
    run at the cold clock); only dir0-critical DMA goes first (x1+x2,
    w1 k/v cols, w2 q cols; dir1 w cols + wo deferred); the first 6
    fill groups pre-accumulate in the 6 idle attention PSUM banks; the
    first attention unit is striped into the kT0/v0 fill chunks.
  - Tail: dir1 qt>=2 units carry 4 dummy matmuls per key block (HAM
    feeder); the final outproj pipelines pops through 2 PSUM banks.
  - Each qt's outproj emission is deferred past the next qt's first
    unit so its pop matmuls run in exp-paced PE slack, not at the qt
    boundary (402 -> ~398us).
  - fp8 does NOT work here: quantizing W_qkv (x32 prescale) gives
    relmax ~0.077, and even fp8 x with on-device bf16 upconvert gives
    ~0.055 (vs 0.02 gate) - the per-channel/per-token quantization
    error is systematic and does not diffuse over the softmax.
  - PSUM pools: st 4 banks, oav 2 (attention-only), ptr/outproj+head
    fills 1, pmm fills 1.
"""

import os

import numpy as np
import ml_dtypes

import concourse.bass as bass
import concourse.mybir as mybir
import concourse.tile as tile
from concourse import bacc
from concourse.bass_utils import run_bass_kernel_spmd
from concourse.masks import make_identity

BF16 = mybir.dt.bfloat16
F16 = mybir.dt.float16
F32 = mybir.dt.float32


B, N, DIM = 2, 2048, 1024
HEADS, DH = 16, 64
HPC = 4                      # heads per core
HC = HPC * DH                # 256 inner columns per core
SCALE = DH ** -0.5
P = 128
FB = DIM // P                # 8 feature blocks
KB = N // P                  # 16 key blocks
QT = 512                     # q-tile
NQT = N // QT                # 4 q-tiles
NM = QT // P                 # 4 m-blocks per q-tile

NCORES = 8
LOWPRI = 10_000_000          # negative high_priority offset for fill work

_NC = None
LAST_RESULTS = None


def _build():
    nc = bacc.Bacc(None, target_bir_lowering=False, debug=False, num_devices=NCORES)

    # x is pre-transposed on the host: x^T [DIM, N] -> linear DMA loads
    xs = [nc.dram_tensor(f"x{s + 1}", [DIM, N], BF16, kind="ExternalInput")
          for s in range(2)]
    ws = [nc.dram_tensor(f"w{s + 1}", [DIM, 3 * HC], BF16, kind="ExternalInput")
          for s in range(2)]
    wos = [nc.dram_tensor(f"wo{s + 1}", [HC, DIM], BF16, kind="ExternalInput")
           for s in range(2)]
    os_ = [nc.dram_tensor(f"o{d + 1}", [N, DIM], F16, kind="ExternalOutput")
           for d in range(2)]

    with tile.TileContext(nc) as tc:
        with (
            tc.tile_pool(name="const", bufs=1) as const_pool,
            tc.tile_pool(name="qkv", bufs=1) as qkv_pool,
        ):
            identity = const_pool.tile([P, P], BF16)
            make_identity(nc, identity[:])
            wo_sb = [const_pool.tile([P, 2, DIM], BF16, name=f"wo{d}")
                     for d in range(2)]

            # persistent per-stream q/k/v (bf16) and per-dir O^T
            qT = [qkv_pool.tile([P, 2, N], BF16, name=f"qT{s}") for s in range(2)]
            kT = [qkv_pool.tile([P, 2, N], BF16, name=f"kT{s}") for s in range(2)]
            vx = [qkv_pool.tile([P, KB, HPC, DH + 1], BF16, name=f"vx{s}")
                  for s in range(2)]
            ot = [qkv_pool.tile([P, 2, N], BF16, name=f"ot{d}") for d in range(2)]
            for s in range(2):
                nc.vector.memset(vx[s][:, :, :, DH], 1.0)

            with (
                tc.tile_pool(name="xT", bufs=1) as xt_pool,
                tc.tile_pool(name="wsb", bufs=1) as w_pool,
                tc.tile_pool(name="pmm", bufs=1, space="PSUM") as pmm_pool,
                tc.tile_pool(name="st", bufs=2, space="PSUM") as st_pool,
                tc.tile_pool(name="oav", bufs=2, space="PSUM") as oav_pool,
                tc.tile_pool(name="ptrpop", bufs=1, space="PSUM") as ptr_pool,
                tc.tile_pool(name="pt", bufs=6) as pt_pool,
                tc.tile_pool(name="osb", bufs=4) as osb_pool,
                tc.tile_pool(name="rec", bufs=4) as rec_pool,
                tc.tile_pool(name="ost", bufs=3) as ost_pool,
            ):
                # HAM warmup: no-dep dummy matmuls, first in the PE queue.
                # The critical x DMA takes ~30us; the first fill group
                # completes only after its full-DIM contraction, so the
                # PE needs dummy coverage through the whole DMA window or
                # the fills run at the cold 1.2 GHz clock.
                warm = pmm_pool.tile([P, P], F32, name="warm", tag="pmm")
                for _ in range(480):
                    nc.tensor.matmul(warm[:], identity[:], identity[:],
                                     start=True, stop=True)

                xT = [xt_pool.tile([P, FB, N], BF16, name=f"xT{s}")
                      for s in range(2)]
                w_sb = [w_pool.tile([P, FB, 3 * HC], BF16, name=f"w{s}")
                        for s in range(2)]
                # Critical DMA first: x of both streams + only the w
                # columns dir0 needs (stream1 k/v, stream2 q). dir1's w
                # columns and wo queue up behind them.
                W_EARLY = [(HC, 3 * HC), (0, HC)]
                W_LATE = [(0, HC), (HC, 3 * HC)]
                for fb in range(FB):
                    for s in (1, 0):
                        lo, hi = W_EARLY[s]
                        nc.sync.dma_start(
                            w_sb[s][:, fb, lo:hi],
                            ws[s][fb * P:(fb + 1) * P, lo:hi])
                        nc.sync.dma_start(
                            xT[s][:, fb, :], xs[s][fb * P:(fb + 1) * P, :])
                for fb in range(FB):
                    for s in (1, 0):
                        lo, hi = W_LATE[s]
                        nc.sync.dma_start(
                            w_sb[s][:, fb, lo:hi],
                            ws[s][fb * P:(fb + 1) * P, lo:hi])
                for d in range(2):
                    for cb in range(2):
                        nc.sync.dma_start(
                            wo_sb[d][:, cb, :], wos[d][cb * P:(cb + 1) * P, :])

                def qk_group(s, off, cb, nt, dest, pool, tag):
                    ps = pool.tile([P, 512], F32, name="pqk", tag=tag)
                    for fb in range(FB):
                        nc.tensor.matmul(
                            ps[:],
                            w_sb[s][:, fb, off + cb * P:off + (cb + 1) * P],
                            xT[s][:, fb, nt * 512:(nt + 1) * 512],
                            start=(fb == 0), stop=(fb == FB - 1))
                    nc.vector.tensor_copy(dest[:, cb, nt * 512:(nt + 1) * 512],
                                          ps[:])

                def v_group(s, kb, pool, tag):
                    ps = pool.tile([P, HC], F32, name="pv", tag=tag)
                    for fb in range(FB):
                        nc.tensor.matmul(
                            ps[:],
                            xT[s][:, fb, kb * P:(kb + 1) * P],
                            w_sb[s][:, fb, 2 * HC:3 * HC],
                            start=(fb == 0), stop=(fb == FB - 1))
                    nc.vector.tensor_copy(
                        vx[s][:, kb, :, 0:DH],
                        ps[:].rearrange("p (h d) -> p h d", h=HPC))

                def attn_begin():
                    return [oav_pool.tile([P, NM, 72], F32, name="oav",
                                          tag="oav")
                            for _ in range(2)]

                def attn_chunk(oav, d, qs, ks, qt, cb, kb_lo, kb_hi,
                               spice=None):
                    q_t, k_t, v_t = qT[qs], kT[ks], vx[ks]
                    q0 = qt * QT
                    for kb in range(kb_lo, kb_hi):
                        st = st_pool.tile([P, 2, QT], F32, name="st", tag="st")
                        for hh in range(2):
                            po = hh * DH
                            nc.tensor.matmul(
                                st[:, hh, :],
                                k_t[po:po + DH, cb, kb * P:(kb + 1) * P],
                                q_t[po:po + DH, cb, q0:q0 + QT],
                                start=True, stop=True)
                        pt = pt_pool.tile([P, 2, QT], BF16, name="pt")
                        nc.scalar.activation(
                            pt[:], st[:],
                            mybir.ActivationFunctionType.Exp, scale=SCALE)
                        for hh in range(2):
                            head = 2 * cb + hh
                            for m in range(NM):
                                nc.tensor.matmul(
                                    oav[hh][:, m, 0:DH + 1],
                                    pt[:, hh, m * P:(m + 1) * P],
                                    v_t[:, kb, head, :],
                                    start=(kb == 0 and m == 0),
                                    stop=(kb == KB - 1 and m == NM - 1),
                                    skip_group_check=True)
                        if spice is not None:
                            # HAM feeder: keep the PE clock warm through
                            # the exp-paced end phase (fills exhausted)
                            for _ in range(4):
                                nc.tensor.matmul(spice[:], identity[:],
                                                 identity[:],
                                                 start=True, stop=True)

                def attn_end(oav, d, qt, cb):
                    # normalize, transpose O -> O^T, write into ot
                    q0 = qt * QT
                    ptr = ptr_pool.tile([DH, 2 * NM, P], BF16, name="ptr",
                                        tag="ptrpop")
                    for hh in range(2):
                        for m in range(NM):
                            rec = rec_pool.tile([P, 1], F32, name="rec")
                            nc.vector.reciprocal(rec[:], oav[hh][:, m, DH:DH + 1])
                            osb = osb_pool.tile([P, DH], BF16, name="osb")
                            nc.vector.tensor_scalar_mul(
                                osb[:], oav[hh][:, m, 0:DH], rec[:])
                            nc.tensor.transpose(
                                ptr[:, hh * NM + m, :], osb[:], identity[:])
                    for hh in range(2):
                        po = hh * DH
                        nc.vector.tensor_copy(
                            ot[d][po:po + DH, cb, q0:q0 + QT],
                            ptr[:, hh * NM:(hh + 1) * NM, :])

                def attn_unit(d, qs, ks, qt, cb):
                    oav = attn_begin()
                    attn_chunk(oav, d, qs, ks, qt, cb, 0, KB)
                    attn_end(oav, d, qt, cb)

                def outproj(d, qt, twobank=False):
                    for mb in range(NM):
                        row = qt * QT + mb * P
                        ost = ost_pool.tile([P, DIM], F16, name="ost")
                        for nb in range(2):
                            # late outprojs: the fill bank is free, use it
                            # to pipeline the pops 2-deep
                            if twobank and (mb * 2 + nb) % 2 == 1:
                                pop = pmm_pool.tile([P, 512], F32, name="pop",
                                                    tag="pmm")
                            else:
                                pop = ptr_pool.tile([P, 512], F32, name="pop",
                                                    tag="ptrpop")
                            for cb2 in range(2):
                                nc.tensor.matmul(
                                    pop[:],
                                    ot[d][:, cb2, row:row + P],
                                    wo_sb[d][:, cb2, nb * 512:(nb + 1) * 512],
                                    start=(cb2 == 0), stop=(cb2 == 1))
                            nc.vector.tensor_copy(ost[:, nb * 512:(nb + 1) * 512],
                                                  pop[:])
                        nc.sync.dma_start(os_[d][row:row + P, :], ost[:])

                # ---- head: dir0's first unit striped into its fills ----
                # Fill groups alternate the pmm/ptrpop banks (2-deep
                # pipeline); st/oav stay attention-only. After each
                # kT0-nt + v0 chunk, the matching kb chunk of unit
                # (dir0, qt0, cb0) is emitted, so exp starts right after
                # the x DMA + ~4 fill groups instead of after all 26.
                hseq = [(pmm_pool, "pmm"), (ptr_pool, "ptrpop")]
                hidx = [0]

                def nxt():
                    pl = hseq[hidx[0] % 2]
                    hidx[0] += 1
                    return pl

                # The 6 PSUM bank-slots that attention will use later
                # (st x2, oav x2) are idle during the DMA window; route
                # the first 6 fill groups through them so all 6
                # pre-accumulate per-fb as x lands, instead of
                # serializing through a 2-bank pipeline afterwards.
                qk_group(1, 0, 0, 0, qT[1], pmm_pool, "pmm")
                qk_group(1, 0, 1, 0, qT[1], ptr_pool, "ptrpop")
                qk_group(0, HC, 0, 0, kT[0], st_pool, "st")
                qk_group(0, HC, 1, 0, kT[0], st_pool, "st")
                v_group(0, 0, oav_pool, "oav")
                v_group(0, 1, oav_pool, "oav")
                oav0 = attn_begin()
                for nt in range(4):
                    if nt > 0:
                        for cb in range(2):
                            pl = nxt()
                            qk_group(0, HC, cb, nt, kT[0], pl[0], pl[1])
                    for kb in range(4 * nt + (2 if nt == 0 else 0),
                                    4 * nt + 4):
                        pl = nxt()
                        v_group(0, kb, pl[0], pl[1])
                    attn_chunk(oav0, 0, 1, 0, 0, 0, 4 * nt, 4 * nt + 4)
                attn_end(oav0, 0, 0, 0)

                # ---- remaining qkv at very low scheduler preference:
                # executes only in PE gaps of the Scalar-bound attention
                # stream. Ordered by first use: dir0 qt1's queries, then
                # dir1 nt0 k/q, dir0 qt2/3 queries, dir1 v + rest.
                with tc.high_priority(offset=-LOWPRI):
                    for cb in range(2):
                        qk_group(1, 0, cb, 1, qT[1], pmm_pool, "pmm")
                    for cb in range(2):
                        qk_group(0, 0, cb, 0, qT[0], pmm_pool, "pmm")
                        qk_group(1, HC, cb, 0, kT[1], pmm_pool, "pmm")
                    for nt in range(2, 4):
                        for cb in range(2):
                            qk_group(1, 0, cb, nt, qT[1], pmm_pool, "pmm")
                    for nt in range(4):
                        for kb in range(4 * nt, 4 * nt + 4):
                            v_group(1, kb, pmm_pool, "pmm")
                        if nt > 0:
                            for cb in range(2):
                                qk_group(1, HC, cb, nt, kT[1], pmm_pool, "pmm")
                    for nt in range(1, 4):
                        for cb in range(2):
                            qk_group(0, 0, cb, nt, qT[0], pmm_pool, "pmm")

                # ---- attention + inline output projection ----
                def spiced(qt, cb):
                    warm2 = pmm_pool.tile([P, P], F32, name="warm2",
                                          tag="pmm")
                    oavL = attn_begin()
                    attn_chunk(oavL, 1, 0, 1, qt, cb, 0, KB, spice=warm2)
                    attn_end(oavL, 1, qt, cb)

                # Each qt's outproj is deferred until after the NEXT
                # qt's first unit: its 8 pop matmuls then execute in
                # that unit's exp-paced PE slack instead of jamming the
                # scalar pipeline at the qt boundary.
                attn_unit(0, 1, 0, 0, 1)
                attn_unit(0, 1, 0, 1, 0)
                outproj(0, 0)
                attn_unit(0, 1, 0, 1, 1)
                attn_unit(0, 1, 0, 2, 0)
                outproj(0, 1)
                attn_unit(0, 1, 0, 2, 1)
                attn_unit(0, 1, 0, 3, 0)
                outproj(0, 2)
                attn_unit(0, 1, 0, 3, 1)
                outproj(0, 3)
                attn_unit(1, 0, 1, 0, 0)
                attn_unit(1, 0, 1, 0, 1)
                attn_unit(1, 0, 1, 1, 0)
                outproj(1, 0)
                attn_unit(1, 0, 1, 1, 1)
                spiced(2, 0)
                outproj(1, 1)
                spiced(2, 1)
                spiced(3, 0)
                outproj(1, 2)
                spiced(3, 1)
                outproj(1, 3, twobank=True)

    nc.compile()
    return nc


def _shard_inputs(x1, x2, W_qkv1, W_qkv2, W_out1, W_out2):
    bf = ml_dtypes.bfloat16
    in_maps = []
    xs = [np.ascontiguousarray(x1).astype(bf), np.ascontiguousarray(x2).astype(bf)]
    w_full = [np.asarray(W_qkv1), np.asarray(W_qkv2)]
    wo_full = [np.asarray(W_out1), np.asarray(W_out2)]
    for cid in range(NCORES):
        b, g = divmod(cid, 4)
        cs = slice(g * HC, (g + 1) * HC)
        m = {}
        for s in range(2):
            m[f"x{s + 1}"] = np.ascontiguousarray(xs[s][b].T)
            w = w_full[s]
            m[f"w{s + 1}"] = np.ascontiguousarray(np.concatenate(
                [w[:, 0:DIM][:, cs], w[:, DIM:2 * DIM][:, cs],
                 w[:, 2 * DIM:3 * DIM][:, cs]], axis=1)).astype(bf)
            m[f"wo{s + 1}"] = np.ascontiguousarray(wo_full[s][cs, :]).astype(bf)
        in_maps.append(m)
    return in_maps


def kernel(x1, x2, W_qkv1, W_qkv2, W_out1, b_out1, W_out2, b_out2):
    global _NC, LAST_RESULTS
    if _NC is None:
        _NC = _build()

    in_maps = _shard_inputs(x1, x2, W_qkv1, W_qkv2, W_out1, W_out2)
    trace = bool(os.environ.get("BASS_KERNEL_TRACE"))
    res = run_bass_kernel_spmd(_NC, in_maps, list(range(NCORES)), trace=trace)
    LAST_RESULTS = res

    out1 = np.zeros((B, N, DIM), np.float32)
    out2 = np.zeros((B, N, DIM), np.float32)
    for cid in range(NCORES):
        b = cid // 4
        out1[b] += res.results[cid]["o1"].astype(np.float32)
        out2[b] += res.results[cid]["o2"].astype(np.float32)
    out1 += np.asarray(b_out1, np.float32)
    out2 += np.asarray(b_out2, np.float32)
    return out1, out2
